# revision 2
# baseline (speedup 1.0000x reference)
"""Trainium2 Bass kernel v2 for the dense transformer block (B=2, T=2048,
C=1024, H=16, D=64, FF=4096), SPMD on 8 NeuronCores.

Sharding: pure data-parallel, zero collectives (as baseline): core cid ->
batch b = cid // 4, rank r = cid % 4; rank owns zigzag chunk pair
{r, 7-r} of 256 tokens each. Each core redundantly computes LN1 + K + V
for its whole batch, then attention/proj/LN2/MLP for its own 512 tokens.

v2 vs baseline:
- bf16 on all matmul operands (fp32 accumulate); residual stream fp32.
  Halves every weight and activation DMA.
- K, V, Q, ln1 stay entirely in SBUF: no DRAM bounce for k/v/q.
- LayerNorm in [c,t] layout: column stats via ones-matmul accumulation,
  -mean/rstd broadcast back via K=1 matmuls read directly from PSUM.
  No PE transposes / PSUM-copy storms; x arrives host-transposed.
- V computed directly in [t, d] layout (ln1T block as lhsT), packed into
  [128, 16*(64+1)] tiles with preset ones column per head: AV matmul gets
  the softmax denominator for free, zero per-head V transposes/copies.
- Scores for s-chunks 0..3 use one 512-wide matmul + exp covering both
  owned q-chunks; s-chunks 4..7 only feed the late q-chunk (256 wide).
- Q packed 2 heads per 128-row block (full PE array).
"""

import numpy as np

B, T, C = 2, 2048, 1024
H, D = 16, 64
FF = 4 * C
EPS = 1e-6
N_CORES = 8
NCHUNK = 8
CH = T // NCHUNK        # 256 tokens per causal chunk
RANKS = 4
OWN = T // RANKS        # 512 tokens owned per core
P = 128
NB = 512                # matmul moving-dim tile
KC = C // P             # 8 contraction chunks over C
TB = T // NB            # 4 column blocks over T
FB = FF // P            # 32 ff row blocks
HB = D + 1              # per-head V columns incl. ones column
MASKED_PAIRS = [(0, sc) for sc in range(4)] + [(1, sc) for sc in range(4, 8)]


def build_core_program(nc, tile, mybir, n_iters=1):
    from contextlib import ExitStack

    dt = mybir.dt
    f32 = dt.float32
    bf16 = dt.bfloat16
    AF = mybir.ActivationFunctionType
    ALU = mybir.AluOpType

    xT_bf = nc.dram_tensor("xT_bf", [C, T], bf16, kind="ExternalInput").ap()
    xoT_bf = nc.dram_tensor("xoT_bf", [C, OWN], bf16,
                            kind="ExternalInput").ap()
    xT_own = nc.dram_tensor("xT_own", [C, OWN], f32, kind="ExternalInput").ap()
    wq_p = nc.dram_tensor("wq_p", [KC, P, KC * P], bf16,
                          kind="ExternalInput").ap()
    wk_p = nc.dram_tensor("wk_p", [KC, P, KC * P], bf16,
                          kind="ExternalInput").ap()
    wv_p = nc.dram_tensor("wv_p", [KC, P, C], bf16, kind="ExternalInput").ap()
    wproj_p = nc.dram_tensor("wproj_p", [KC, P, KC * P], bf16,
                             kind="ExternalInput").ap()
    wl1_p = nc.dram_tensor("wl1_p", [FB, P, KC * P], bf16,
                           kind="ExternalInput").ap()
    wl3_p = nc.dram_tensor("wl3_p", [KC, P, FB * P], bf16,
                           kind="ExternalInput").ap()
    bqkv = nc.dram_tensor("bqkv", [3 * C], f32, kind="ExternalInput").ap()
    bv_bf = nc.dram_tensor("bv_bf", [1, C], bf16, kind="ExternalInput").ap()
    bproj = nc.dram_tensor("bproj", [C], f32, kind="ExternalInput").ap()
    bl1 = nc.dram_tensor("bl1", [FF], f32, kind="ExternalInput").ap()
    bl3 = nc.dram_tensor("bl3", [C], f32, kind="ExternalInput").ap()
    mask_in = nc.dram_tensor(
        "mask", [len(MASKED_PAIRS), 2, P, CH], bf16, kind="ExternalInput"
    ).ap()
    outT = nc.dram_tensor("outT", [C, OWN], f32, kind="ExternalOutput").ap()

    with tile.TileContext(nc) as tc, ExitStack() as ctx:
        cpool = ctx.enter_context(tc.tile_pool(name="const", bufs=1))
        onesf = cpool.tile([P, 1], bf16, name="onesf")
        nc.vector.memset(onesf[:], 1.0)
        ones1 = cpool.tile([1, D], bf16, name="ones1")
        nc.vector.memset(ones1[:], 1.0)
        onesr = cpool.tile([1, P], bf16, name="onesr")
        nc.vector.memset(onesr[:], 1.0)
        onesf32 = cpool.tile([P, 1], f32, name="onesf32")
        nc.vector.memset(onesf32[:], 1.0)

        bqkv_t = cpool.tile([P, 3 * KC], f32, name="bqkv_t")
        nc.sync.dma_start(bqkv_t[:], bqkv.rearrange("(j p) -> p j", p=P))
        bv_s = cpool.tile([1, C], bf16, name="bv_s")
        nc.sync.dma_start(bv_s[:], bv_bf)
        bproj_t = cpool.tile([P, KC], f32, name="bproj_t")
        nc.sync.dma_start(bproj_t[:], bproj.rearrange("(j p) -> p j", p=P))
        bl1_t = cpool.tile([P, FB], f32, name="bl1_t")
        nc.sync.dma_start(bl1_t[:], bl1.rearrange("(j p) -> p j", p=P))
        bl3_t = cpool.tile([P, KC], f32, name="bl3_t")
        nc.sync.dma_start(bl3_t[:], bl3.rearrange("(j p) -> p j", p=P))


        def mm(out, lhsT, rhs, **kw):
            nc.tensor.matmul(out, lhsT, rhs, **kw)

        def ln_normalize_ct(src_tiles, nb0, ncols, dst_tiles, dnb0,
                            ps_stat, ps_bc, work, zwork, stat,
                            src_f32=False):
            """LN over channel dim for token-columns [nb0, nb0+ncols) of the
            [c,t] tiles src_tiles; writes bf16 normalized output into
            dst_tiles at column dnb0. Stats via ones-matmuls; broadcast
            (-mu, rstd) via K=1 matmuls, consumed directly from PSUM."""
            ocol = onesf32 if src_f32 else onesf
            s_ps = ps_stat.tile([1, NB], f32, name="s_ps")
            for kc in range(KC):
                mm(s_ps[0:1, 0:ncols], ocol[:, 0:1],
                   src_tiles[kc][:, nb0:nb0 + ncols],
                   start=(kc == 0), stop=(kc == KC - 1))
            sq_ps = ps_stat.tile([1, NB], f32, name="sq_ps")
            for kc in range(KC):
                sq = work.tile([P, NB], bf16, name="sq")
                nc.scalar.activation(sq[:, 0:ncols],
                                     src_tiles[kc][:, nb0:nb0 + ncols],
                                     AF.Square)
                mm(sq_ps[0:1, 0:ncols], onesf[:, 0:1], sq[:, 0:ncols],
                   start=(kc == 0), stop=(kc == KC - 1))
            nmu = stat.tile([1, NB], bf16, name="nmu")
            nc.scalar.mul(nmu[0:1, 0:ncols], s_ps[0:1, 0:ncols], -1.0 / C)
            s2n = stat.tile([1, NB], f32, name="s2n")
            nc.vector.tensor_mul(s2n[0:1, 0:ncols], nmu[0:1, 0:ncols],
                                 s_ps[0:1, 0:ncols])
            varn = stat.tile([1, NB], f32, name="varn")
            nc.vector.tensor_add(varn[0:1, 0:ncols], sq_ps[0:1, 0:ncols],
                                 s2n[0:1, 0:ncols])
            sd = stat.tile([1, NB], f32, name="sd")
            nc.scalar.activation(sd[0:1, 0:ncols], varn[0:1, 0:ncols],
                                 AF.Sqrt, scale=1.0 / (C - 1))
            sde = stat.tile([1, NB], f32, name="sde")
            nc.vector.tensor_scalar_add(sde[0:1, 0:ncols],
                                        sd[0:1, 0:ncols], EPS)
            rstd = stat.tile([1, NB], bf16, name="rstd")
            with nc.allow_low_precision(reason="rstd bf16 for bcast matmul"):
                nc.vector.reciprocal(rstd[0:1, 0:ncols], sde[0:1, 0:ncols])
            mu_ps = ps_bc.tile([P, NB], f32, name="mu_ps")
            mm(mu_ps[:, 0:ncols], onesr[:], nmu[0:1, 0:ncols],
               start=True, stop=True)
            rs_ps = ps_bc.tile([P, NB], f32, name="rs_ps")
            mm(rs_ps[:, 0:ncols], onesr[:], rstd[0:1, 0:ncols],
               start=True, stop=True)
            for kc in range(KC):
                d = zwork.tile([P, NB], bf16, name="d")
                nc.vector.tensor_add(d[:, 0:ncols],
                                     src_tiles[kc][:, nb0:nb0 + ncols],
                                     mu_ps[:, 0:ncols])
                nc.vector.tensor_mul(
                    dst_tiles[kc][:, dnb0:dnb0 + ncols],
                    d[:, 0:ncols], rs_ps[:, 0:ncols])

        def attn_block(ctx3, aT):
            vpkp = ctx3.enter_context(tc.tile_pool(name="vpkp", bufs=1))
            vpk = [vpkp.tile([P, H * HB], bf16, name=f"vpk{j}")
                   for j in range(T // P)]
            kTp = ctx3.enter_context(tc.tile_pool(name="kTp", bufs=1))
            kT = [kTp.tile([P, T], bf16, name=f"kT{j}") for j in range(KC)]
            qTp = ctx3.enter_context(tc.tile_pool(name="qTp", bufs=1))
            qT = [qTp.tile([P, OWN], bf16, name=f"qT{j}") for j in range(KC)]

            # ---- A/B: LN1 (in place), V, K ----
            with tc.tile_pool(name="ln1p", bufs=1) as ln1p, \
                 tc.tile_pool(name="stata", bufs=4) as stata, \
                 tc.tile_pool(name="worka", bufs=3) as worka, \
                 tc.tile_pool(name="zwork", bufs=2) as zwork, \
                 tc.tile_pool(name="psst", bufs=1, space="PSUM") as ps_stat, \
                 tc.tile_pool(name="psbc", bufs=1, space="PSUM") as ps_bc, \
                 tc.tile_pool(name="psmm", bufs=3, space="PSUM") as psmm:
                ln1T = [ln1p.tile([P, T], bf16, name=f"ln1T{j}")
                        for j in range(KC)]
                for kc in range(KC):
                    nc.sync.dma_start(ln1T[kc][:],
                                      xT_bf[kc * P:(kc + 1) * P, :])
                for nb in range(TB):
                    ln_normalize_ct(ln1T, nb * NB, NB, ln1T, nb * NB,
                                    ps_stat, ps_bc, worka, zwork, stata)

                # V direct in [t, d]: lhsT = ln1T t-slice, rhs = wv columns
                for tb in range(T // P):
                    nc.vector.memset(
                        vpk[tb][:].rearrange(
                            "p (h x) -> p h x", h=H)[:, :, D:D + 1], 1.0)
                with tc.tile_pool(name="wvp", bufs=2) as wvp:
                    for half in range(2):
                        wvh = wvp.tile([P, KC * NB], bf16, name="wvh")
                        for kc in range(KC):
                            nc.sync.dma_start(
                                wvh[:, kc * NB:(kc + 1) * NB],
                                wv_p[kc, :, half * NB:(half + 1) * NB])
                        bvb = wvp.tile([P, NB], bf16, name="bvb")
                        bps = psmm.tile([P, NB], f32, name="ps")
                        mm(bps[:], onesr[:],
                           bv_s[0:1, half * NB:(half + 1) * NB],
                           start=True, stop=True)
                        nc.vector.tensor_copy(bvb[:], bps[:])
                        for tb in range(T // P):
                            ps = psmm.tile([P, NB], f32, name="ps")
                            for kc in range(KC):
                                mm(ps[:],
                                   ln1T[kc][:, tb * P:(tb + 1) * P],
                                   wvh[:, kc * NB:(kc + 1) * NB],
                                   start=(kc == 0), stop=(kc == KC - 1))
                            nc.vector.tensor_add(
                                vpk[tb][:].rearrange(
                                    "p (h x) -> p h x", h=H
                                )[:, half * 8:(half + 1) * 8, 0:D],
                                ps[:].rearrange("p (h d) -> p h d", d=D),
                                bvb[:].rearrange("p (h d) -> p h d", d=D))

                # K^T row-blocks (2 heads per 128-block)
                with tc.tile_pool(name="wkp", bufs=3) as wkp:
                    for m in range(KC):
                        wk = wkp.tile([P, KC * P], bf16, name="wk")
                        nc.sync.dma_start(wk[:], wk_p[m])
                        for nb in range(TB):
                            ps = psmm.tile([P, NB], f32, name="ps")
                            for kc in range(KC):
                                mm(ps[:], wk[:, kc * P:(kc + 1) * P],
                                   ln1T[kc][:, nb * NB:(nb + 1) * NB],
                                   start=(kc == 0), stop=(kc == KC - 1))
                            nc.vector.tensor_scalar_add(
                                kT[m][:, nb * NB:(nb + 1) * NB], ps[:],
                                bqkv_t[:, KC + m:KC + m + 1])

            # ---- C: LN1 on own tokens (in place) -> ln1q; then Q ----
            with tc.tile_pool(name="lnqp", bufs=1) as lnqp, \
                 tc.tile_pool(name="statq", bufs=4) as statq, \
                 tc.tile_pool(name="workq", bufs=3) as workq, \
                 tc.tile_pool(name="zwkq", bufs=2) as zwkq, \
                 tc.tile_pool(name="wqp", bufs=3) as wqp, \
                 tc.tile_pool(name="psq1", bufs=1, space="PSUM") as ps_sq, \
                 tc.tile_pool(name="psq2", bufs=1, space="PSUM") as ps_bq, \
                 tc.tile_pool(name="psq3", bufs=3, space="PSUM") as psmq:
                ln1q = [lnqp.tile([P, OWN], bf16, name=f"ln1q{j}")
                        for j in range(KC)]
                for kc in range(KC):
                    nc.sync.dma_start(ln1q[kc][:],
                                      xoT_bf[kc * P:(kc + 1) * P, :])
                ln_normalize_ct(ln1q, 0, NB, ln1q, 0,
                                ps_sq, ps_bq, workq, zwkq, statq)
                for m in range(KC):
                    wq = wqp.tile([P, KC * P], bf16, name="wq")
                    nc.sync.dma_start(wq[:], wq_p[m])
                    ps = psmq.tile([P, NB], f32, name="ps")
                    for kc in range(KC):
                        mm(ps[:], wq[:, kc * P:(kc + 1) * P], ln1q[kc][:],
                           start=(kc == 0), stop=(kc == KC - 1))
                    nc.scalar.activation(qT[m][:], ps[:], AF.Identity,
                                         bias=bqkv_t[:, m:m + 1])

            # ---- D: attention -> aT ----
            with tc.tile_pool(name="maskp", bufs=1) as mpool, \
                 tc.tile_pool(name="statd", bufs=4) as statd, \
                 tc.tile_pool(name="exp0", bufs=4) as exp0, \
                 tc.tile_pool(name="exp1", bufs=4) as exp1, \
                 tc.tile_pool(name="bcsp", bufs=4) as bcsp, \
                 tc.tile_pool(name="pssc", bufs=2, space="PSUM") as ps_sc, \
                 tc.tile_pool(name="psav", bufs=2, space="PSUM") as ps_av, \
                 tc.tile_pool(name="psbz", bufs=2, space="PSUM") as ps_bz:
                mtiles = {}
                for i, (qh, sc) in enumerate(MASKED_PAIRS):
                    for sb in range(2):
                        mt = mpool.tile([P, CH], bf16, name=f"m{qh}_{sc}_{sb}")
                        nc.sync.dma_start(mt[:], mask_in[i, sb])
                        mtiles[(qh, sc, sb)] = mt

                for h in range(H):
                    hr = (h % 2) * D          # partition offset within block
                    kt = kT[h // 2]
                    qt = qT[h // 2]
                    av0 = ps_av.tile([HB, CH], f32, name="av0")
                    av1 = ps_av.tile([HB, CH], f32, name="av1")
                    for sc in range(4):
                        for sb in range(2):
                            blk = sc * 2 + sb
                            s0 = blk * P
                            vsl = vpk[blk][:].rearrange(
                                "p (hh x) -> p hh x", hh=H)[:, h, :]
                            ps = ps_sc.tile([P, NB], f32, name="ps")
                            mm(ps[:], kt[hr:hr + D, s0:s0 + P],
                               qt[hr:hr + D, :], start=True, stop=True)
                            ex = exp0.tile([P, NB], bf16, name="ex")
                            nc.scalar.activation(ex[:], ps[:], AF.Exp)
                            exm = exp1.tile([P, CH], bf16, name="exm")
                            nc.gpsimd.tensor_mul(
                                exm[:], ex[:, 0:CH], mtiles[(0, sc, sb)][:])
                            mm(av0[:], vsl, exm[:],
                               start=(blk == 0), stop=(blk == 7))
                            mm(av1[:], vsl, ex[:, CH:NB],
                               start=(blk == 0), stop=(blk == 15))
                    for scp in (4, 6):
                        # paired: scores for chunks scp, scp+1 -> one psum
                        # per chunk-half pair, one 512-wide exp each
                        for pp in range(2):
                            scc = scp + pp
                            ps = ps_sc.tile([P, NB], f32, name="ps")
                            for sb in range(2):
                                s0 = (scc * 2 + sb) * P
                                mm(ps[:, sb * CH:(sb + 1) * CH],
                                   kt[hr:hr + D, s0:s0 + P],
                                   qt[hr:hr + D, CH:NB],
                                   start=True, stop=True)
                            ex = exp0.tile([P, NB], bf16, name="ex")
                            nc.scalar.activation(ex[:], ps[:], AF.Exp)
                            for sb in range(2):
                                blk = scc * 2 + sb
                                vsl = vpk[blk][:].rearrange(
                                    "p (hh x) -> p hh x", hh=H)[:, h, :]
                                exm = exp1.tile([P, CH], bf16, name="exm")
                                nc.gpsimd.tensor_mul(
                                    exm[:], ex[:, sb * CH:(sb + 1) * CH],
                                    mtiles[(1, scc, sb)][:])
                                mm(av1[:], vsl, exm[:],
                                   start=False, stop=(blk == 15))
                    for qh, av in ((0, av0), (1, av1)):
                        rz = statd.tile([1, CH], f32, name="rz")
                        nc.vector.reciprocal(rz[:], av[D:D + 1, :])
                        rzr = statd.tile([1, CH], bf16, name="rzr")
                        nc.scalar.activation(rzr[:], rz[:], AF.Identity)
                        bc = ps_bz.tile([D, CH], f32, name="bc")
                        mm(bc[:], ones1[:], rzr[:], start=True, stop=True)
                        bcs = bcsp.tile([D, CH], bf16, name="bcs")
                        nc.vector.tensor_copy(bcs[:], bc[:])
                        nc.vector.tensor_mul(
                            aT[h // 2][hr:hr + D, qh * CH:(qh + 1) * CH],
                            av[0:D, :], bcs[:])

        def body(ctx2):
            from contextlib import ExitStack as _ES
            aTp = ctx2.enter_context(tc.tile_pool(name="aTp", bufs=1))
            aT = [aTp.tile([P, OWN], bf16, name=f"aT{j}") for j in range(KC)]
            with _ES() as ctx3:
                attn_block(ctx3, aT)

            # ---- E: proj + residual -> x1T (fp32) ----
            x1Tp = ctx2.enter_context(tc.tile_pool(name="x1Tp", bufs=1))
            x1T = [x1Tp.tile([P, OWN], f32, name=f"x1T{j}") for j in range(KC)]
            with tc.tile_pool(name="wpp", bufs=3) as wpp, \
                 tc.tile_pool(name="eve", bufs=6) as evp, \
                 tc.tile_pool(name="psm2", bufs=3, space="PSUM") as psm2:
                for m in range(KC):
                    wp = wpp.tile([P, KC * P], bf16, name="wp")
                    nc.sync.dma_start(wp[:], wproj_p[m])
                    ps = psm2.tile([P, NB], f32, name="ps")
                    for kc in range(KC):
                        mm(ps[:], wp[:, kc * P:(kc + 1) * P], aT[kc][:],
                           start=(kc == 0), stop=(kc == KC - 1))
                    ev = evp.tile([P, NB], f32, name="ev")
                    nc.vector.tensor_scalar_add(ev[:], ps[:],
                                                bproj_t[:, m:m + 1])
                    xo = evp.tile([P, NB], f32, name="xo2")
                    nc.sync.dma_start(xo[:], xT_own[m * P:(m + 1) * P, :])
                    nc.vector.tensor_add(x1T[m][:], ev[:], xo[:])

            # ---- F/G: LN2 in [c,t], then MLP ----
            with tc.tile_pool(name="ln2p", bufs=1) as ln2p, \
                 tc.tile_pool(name="statf", bufs=4) as statf, \
                 tc.tile_pool(name="workf", bufs=4) as workf, \
                 tc.tile_pool(name="zwk2", bufs=3) as zwk2, \
                 tc.tile_pool(name="pss2", bufs=1, space="PSUM") as ps_st2, \
                 tc.tile_pool(name="psb2", bufs=1, space="PSUM") as ps_bc2, \
                 tc.tile_pool(name="psm3", bufs=3, space="PSUM") as psm3:
                ln2T = [ln2p.tile([P, OWN], bf16, name=f"ln2T{j}")
                        for j in range(KC)]
                ln_normalize_ct(x1T, 0, NB, ln2T, 0,
                                ps_st2, ps_bc2, workf, zwk2, statf,
                                src_f32=True)
                with tc.tile_pool(name="hTp", bufs=1) as hTp, \
                     tc.tile_pool(name="evg", bufs=6) as evp, \
                     tc.tile_pool(name="w1p", bufs=3) as w1p:
                    hT = [hTp.tile([P, OWN], bf16, name=f"hT{j}")
                          for j in range(FB)]
                    for m in range(FB):
                        w1 = w1p.tile([P, KC * P], bf16, name="w1")
                        nc.sync.dma_start(w1[:], wl1_p[m])
                        ps = psm3.tile([P, NB], f32, name="ps")
                        for kc in range(KC):
                            mm(ps[:], w1[:, kc * P:(kc + 1) * P], ln2T[kc][:],
                               start=(kc == 0), stop=(kc == KC - 1))
                        nc.vector.tensor_scalar(
                            hT[m][:], ps[:], bl1_t[:, m:m + 1], 0.0,
                            ALU.add, ALU.max)
                    with tc.tile_pool(name="w3p", bufs=2) as w3p:
                        for m in range(KC):
                            w3 = w3p.tile([P, FB * P], bf16, name="w3")
                            nc.sync.dma_start(w3[:], wl3_p[m])
                            psa = psm3.tile([P, NB], f32, name="ps")
                            for fc in range(FB // 2):
                                mm(psa[:], w3[:, fc * P:(fc + 1) * P],
                                   hT[fc][:],
                                   start=(fc == 0), stop=(fc == FB // 2 - 1))
                            eva = evp.tile([P, NB], f32, name="ev")
                            nc.vector.tensor_scalar_add(
                                eva[:], psa[:], bl3_t[:, m:m + 1])
                            psb = psm3.tile([P, NB], f32, name="ps")
                            for fc in range(FB // 2, FB):
                                mm(psb[:], w3[:, fc * P:(fc + 1) * P],
                                   hT[fc][:],
                                   start=(fc == FB // 2), stop=(fc == FB - 1))
                            evb = evp.tile([P, NB], f32, name="ev")
                            nc.vector.tensor_add(evb[:], psb[:], x1T[m][:])
                            o = evp.tile([P, NB], f32, name="o", tag="ev")
                            nc.vector.tensor_add(o[:], eva[:], evb[:])
                            nc.sync.dma_start(outT[m * P:(m + 1) * P, :],
                                              o[:])

        if n_iters == 1:
            with ExitStack() as ctx2:
                body(ctx2)
        else:
            with tc.For_i(0, n_iters, 1):
                with ExitStack() as ctx2:
                    body(ctx2)

    return nc


def _pack(wT, nblk, blk):
    """Pack lhsT source wT [K_total, M_total] into [nblk, P, (K/P)*blk]
    tiles: packed[m, p, kc*blk + j] = wT[kc*P + p, m*blk + j]."""
    K_total, M_total = wT.shape
    kc = K_total // P
    assert M_total == nblk * blk
    return np.ascontiguousarray(
        wT.reshape(kc, P, nblk, blk).transpose(2, 1, 0, 3)
        .reshape(nblk, P, kc * blk)
    )


def _host_prep(x, qkv_w, proj_w, proj_b, l1_w, l1_b, l3_w, l3_b,
               ln1_g, ln1_b, ln2_g, ln2_b):
    import ml_dtypes
    f = np.float32
    bf = ml_dtypes.bfloat16
    x = np.asarray(x, f)
    qkv_w = np.asarray(qkv_w, f)
    scale = np.float32(D ** -0.5)
    w_eff = qkv_w * np.asarray(ln1_g, f)[None, :]
    b_eff = (qkv_w @ np.asarray(ln1_b, f)).astype(f)
    w_eff[:C] *= scale
    b_eff[:C] *= scale
    l1_eff = np.asarray(l1_w, f) * np.asarray(ln2_g, f)[None, :]
    bl1_eff = (np.asarray(l1_b, f)
               + np.asarray(l1_w, f) @ np.asarray(ln2_b, f)).astype(f)
    wT = np.ascontiguousarray(w_eff.T)       # [cin, 3C]
    shared = {
        "wq_p": _pack(wT[:, 0:C], KC, P).astype(bf),
        "wk_p": _pack(wT[:, C:2 * C], KC, P).astype(bf),
        "wv_p": np.ascontiguousarray(
            wT[:, 2 * C:3 * C].reshape(KC, P, C)).astype(bf),
        "wproj_p": _pack(np.ascontiguousarray(np.asarray(proj_w, f).T),
                         KC, P).astype(bf),
        "wl1_p": _pack(np.ascontiguousarray(l1_eff.T), FB, P).astype(bf),
        "wl3_p": _pack(np.ascontiguousarray(np.asarray(l3_w, f).T),
                       KC, P).astype(bf),
        "bqkv": b_eff,
        "bv_bf": b_eff[2 * C:3 * C].reshape(1, C).astype(bf),
        "bproj": np.asarray(proj_b, f),
        "bl1": bl1_eff,
        "bl3": np.asarray(l3_b, f),
    }

    in_maps = []
    for cid in range(N_CORES):
        b, r = divmod(cid, RANKS)
        lo, hi = r, NCHUNK - 1 - r
        own_idx = np.r_[lo * CH:(lo + 1) * CH, hi * CH:(hi + 1) * CH]
        xb = x[b]
        x_own = np.ascontiguousarray(xb[own_idx])
        m = np.zeros((len(MASKED_PAIRS), 2, P, CH), f)
        tri0 = (np.arange(P)[:, None] <= np.arange(CH)[None, :]).astype(f)
        tri1 = (np.arange(P)[:, None] + P <= np.arange(CH)[None, :]).astype(f)
        for i, (qh, sc) in enumerate(MASKED_PAIRS):
            qc = lo if qh == 0 else hi
            if sc < qc:
                m[i] = 1.0
            elif sc == qc:
                m[i, 0] = tri0
                m[i, 1] = tri1
        in_maps.append({
            "xT_bf": np.ascontiguousarray(xb.T).astype(bf),
            "xoT_bf": np.ascontiguousarray(x_own.T).astype(bf),
            "xT_own": np.ascontiguousarray(x_own.T),
            "mask": m.astype(bf),
            **shared,
        })
    return in_maps


def _assemble(results):
    out = np.empty((B, T, C), np.float32)
    for cid in range(N_CORES):
        b, r = divmod(cid, RANKS)
        lo, hi = r, NCHUNK - 1 - r
        oT = results[cid]["outT"]
        out[b, lo * CH:(lo + 1) * CH] = oT[:, 0:CH].T
        out[b, hi * CH:(hi + 1) * CH] = oT[:, CH:2 * CH].T
    return out


_CACHE = {}


def get_nc(n_iters=1):
    if n_iters not in _CACHE:
        import concourse.bacc as bacc
        import concourse.tile as tile
        from concourse import mybir
        nc = bacc.Bacc("TRN2", target_bir_lowering=False, debug=False,
                       num_devices=N_CORES)
        build_core_program(nc, tile, mybir, n_iters=n_iters)
        nc.compile()
        _CACHE[n_iters] = nc
    return _CACHE[n_iters]


def run(inputs, n_iters=1):
    from concourse.bass_utils import run_bass_kernel_spmd
    in_maps = _host_prep(**inputs)
    nc = get_nc(n_iters)
    res = run_bass_kernel_spmd(nc, in_maps, list(range(N_CORES)))
    return _assemble(res.results)


def kernel(**inputs):
    return run(inputs, n_iters=1)
